# revision 16
# baseline (speedup 1.0000x reference)
"""Trainium2 Bass kernel for nn_Encoder_506806141403.

12-layer transformer encoder (D=768, H=12, FF=3072) with the quirk that
attention scores use Q vs V (no K projection) and scale by D**-0.5.

Sharding: 8 cores = 4 batch elements x 2 sequence halves. Each core owns
512 query rows of one batch element. Per layer, each core computes its half's
V projection (token-major, bf16) and the pair exchanges halves via a 2-rank
AllGather, so every core holds all 1024 keys/values of its batch element.
Everything else (LN, Q, scores, softmax, ctx, Wo, FFN) is computed per-core
on its own 512 rows. Matmuls run in float32r (full-rate fp32 on the PE) for
the residual/FFN path and bf16 for the attention-probability path.

Layout notes:
- Residual stream is token-major f32r SBUF tiles [128, 4, 768].
- Matmul inputs are feature-major (contraction dim on partitions); activations
  are transposed via PE-transpose where needed.
- Softmax runs on key-major scoresT [k, q] psum; Z is obtained by augmenting
  the token-major V with a ones column (M=65 matmuls); normalization uses a
  reciprocal + gpsimd partition_broadcast + one DVE multiply per head.
- LN gains/biases are folded into the following projection weights on the
  host; remaining per-channel biases use per-partition ACT bias (feature-
  major outputs) or K=1 outer-product matmuls (token-major outputs).
"""
import os
import sys

sys.path.insert(0, "/opt/trn_rl_repo")

import numpy as np
import ml_dtypes

import concourse.bass as bass
from concourse import bacc
import concourse.tile as tile
from concourse import mybir
from concourse.bass_utils import run_bass_kernel_spmd
from concourse import bass_utils as _bu

if int(os.environ.get("KERNEL_LDW_OPT", "0")):
    _orig_run_command = _bu.run_command

    def _patched_run_command(argv, **kwargs):
        argv = [a.replace("--enable-ldw-opt=false", "--enable-ldw-opt=true")
                if isinstance(a, str) else a for a in argv]
        return _orig_run_command(argv, **kwargs)

    _bu.run_command = _patched_run_command

P = 128
D = 768
H = 12
DH = 64
FF = 3072
NB_D = 6          # D / P
NB_T = 4          # own tokens 512 / P
NB_K = 8          # full tokens 1024 / P
NB_FF = 24        # FF / P
T_OWN = 512
SCALE = float(D) ** -0.5
LN_EPS = 1e-5
N_LAYERS = int(os.environ.get("KERNEL_N_LAYERS", "12"))

F32 = mybir.dt.float32
F32R = mybir.dt.float32r
BF16 = mybir.dt.bfloat16
AF = mybir.ActivationFunctionType
OP = mybir.AluOpType

REPLICA_GROUPS = [[0, 1], [2, 3], [4, 5], [6, 7]]

_cached = {}
_last_results = None


def _register_ntff_hook():
    """Register the axon NTFF profile hook (for trace=True exec timing)."""
    import types
    try:
        import antenv.axon_hooks  # noqa: F401
        return
    except ImportError:
        pass
    try:
        from trn_agent_boot.trn_boot import _ntff_profile_via_ctypes
        import antenv
        hook = _ntff_profile_via_ctypes("/opt/axon/libaxon_pjrt.so")
        mod = types.ModuleType("antenv.axon_hooks")
        mod.get_axon_ntff_profile_hook = lambda: hook
        mod.set_axon_ntff_profile_hook = lambda h: None
        sys.modules["antenv.axon_hooks"] = mod
        antenv.axon_hooks = mod
    except Exception:
        pass


def _regions():
    return ((0, 512), (512, 768))


def build(n_layers=N_LAYERS):
    nc = bacc.Bacc(None, target_bir_lowering=False, num_devices=8)
    L = n_layers

    x_d = nc.dram_tensor("x", [P, NB_T, D], F32R, kind="ExternalInput")
    wq_d = nc.dram_tensor("wq", [L, P, NB_D * D], BF16, kind="ExternalInput")
    wv_d = nc.dram_tensor("wv", [L, P, NB_D * D], BF16, kind="ExternalInput")
    wo_d = nc.dram_tensor("wo", [L, P, NB_D * D], BF16, kind="ExternalInput")
    w1_d = nc.dram_tensor("w1", [L, 4, P, NB_D * D], BF16, kind="ExternalInput")
    w2_d = nc.dram_tensor("w2", [L, 4, P, NB_D * D], BF16, kind="ExternalInput")
    bq_d = nc.dram_tensor("bq", [P, L, NB_D], F32, kind="ExternalInput")
    b1_d = nc.dram_tensor("b1", [P, L, NB_FF], F32, kind="ExternalInput")
    bvr_d = nc.dram_tensor("bv_row", [1, L, D], F32, kind="ExternalInput")
    bor_d = nc.dram_tensor("bo_row", [1, L, D], BF16, kind="ExternalInput")
    b2r_d = nc.dram_tensor("b2_row", [1, L, D], BF16, kind="ExternalInput")
    id32_d = nc.dram_tensor("ident32", [P, P], F32R, kind="ExternalInput")
    idbf_d = nc.dram_tensor("identbf", [P, P], BF16, kind="ExternalInput")
    ones_d = nc.dram_tensor("ones1", [1, P], BF16, kind="ExternalInput")
    out_d = nc.dram_tensor("out", [P, NB_T, D], F32R, kind="ExternalOutput")

    with tile.TileContext(nc) as tc:
        with (
            tc.tile_pool(name="state", bufs=1) as st,
            tc.tile_pool(name="stream", bufs=2) as sp,
            tc.tile_pool(name="acts", bufs=1) as ap,
            tc.tile_pool(name="wpool", bufs=2) as wp,
            tc.tile_pool(name="psA", bufs=2, space="PSUM") as psA,
            tc.tile_pool(name="psB", bufs=4, space="PSUM") as psB,
            tc.tile_pool(name="dram", bufs=2, space="DRAM") as dp,
        ):
            # ---- constants ----
            id32 = st.tile([P, P], F32R)
            idbf = st.tile([P, P], BF16)
            ones1 = st.tile([1, P], BF16)
            bq_all = st.tile([P, L, NB_D], F32)
            b1_all = st.tile([P, L, NB_FF], F32)
            eps_t = st.tile([P, 1], F32)
            q_par = [st.tile([P, NB_D, T_OWN], BF16, name=f"qpar{p}") for p in range(2)]
            nc.vector.memset(q_par[0][:], 0.0)
            nc.vector.memset(q_par[1][:], 0.0)
            nc.sync.dma_start(id32[:], id32_d[:])
            nc.sync.dma_start(idbf[:], idbf_d[:])
            nc.sync.dma_start(ones1[:], ones_d[:])
            nc.sync.dma_start(bq_all[:], bq_d[:])
            nc.sync.dma_start(b1_all[:], b1_d[:])
            nc.vector.memset(eps_t[:], LN_EPS)

            def ln(out_ap, in_ap):
                """LayerNorm (normalize only) along free axis of [128, 768]."""
                t = ap.tile([P, 32], F32, tag="lnscratch", bufs=3, name="lnt")
                stt = t[:, 0:18].rearrange("p (g s) -> p g s", s=6)
                xg = in_ap.rearrange("p (g d) -> p g d", g=3)
                for g in range(3):
                    nc.vector.bn_stats(stt[:, g, :], xg[:, g, :])
                mv = t[:, 18:20]
                nc.vector.bn_aggr(mv[:], stt[:])
                nc.scalar.activation(t[:, 20:21], mv[:, 1:2], AF.Ln, bias=eps_t[:], scale=1.0)
                nc.scalar.activation(t[:, 21:22], t[:, 20:21], AF.Exp, scale=-0.5)
                nc.vector.tensor_scalar(
                    out=out_ap, in0=in_ap, scalar1=mv[:, 0:1], scalar2=t[:, 21:22],
                    op0=OP.subtract, op1=OP.mult,
                )

            def transpose_tm_to_fm(h_tm, name):
                """[128, 4, 768] token-major -> [128, 6, 512] feature-major."""
                h_fm = ap.tile([P, NB_D, T_OWN], BF16, tag="h_fm", bufs=1, name=name)
                for db in range(NB_D):
                    trp = psB.tile([P, T_OWN], F32R, tag="psB", name=f"{name}_tr{db}")
                    for tb in range(NB_T):
                        nc.tensor.transpose(
                            trp[:, tb * P:(tb + 1) * P],
                            h_tm[:, tb, db * P:(db + 1) * P], id32[:],
                        )
                    nc.vector.tensor_copy(h_fm[:, db, :], trp[:])
                return h_fm

            # ---- initial stream ----
            x_t = sp.tile([P, NB_T, D], F32R, tag="stream", name="x0")
            nc.sync.dma_start(x_t[:], x_d[:])

            for l in range(L):
                # ---- weights for this layer ----
                wq = wp.tile([P, NB_D, D], BF16, tag="wbf", bufs=5, name=f"wq{l}")
                nc.sync.dma_start(wq[:], wq_d[l].rearrange("p (k n) -> p k n", n=D))
                wv = wp.tile([P, NB_D, D], BF16, tag="wbf", bufs=5, name=f"wv{l}")
                nc.sync.dma_start(wv[:], wv_d[l].rearrange("p (k n) -> p k n", n=D))
                bv_row = ap.tile([1, D], F32, tag="bvrow", bufs=2, name=f"bvr{l}")
                nc.sync.dma_start(bv_row[:], bvr_d[:, l, :])
                bo_row = ap.tile([1, D], BF16, tag="borow", bufs=2, name=f"bor{l}")
                nc.sync.dma_start(bo_row[:], bor_d[:, l, :])
                b2_row = ap.tile([1, D], BF16, tag="b2row", bufs=2, name=f"b2r{l}")
                nc.sync.dma_start(b2_row[:], b2r_d[:, l, :])

                # ---- LN1 + transpose ----
                with nc.named_scope(f"L{l:02d}_a_ln1"):
                    h_tm = ap.tile([P, NB_T, D], F32R, tag="h_tm", bufs=1, name=f"h1tm{l}")
                    for tb in range(NB_T):
                        ln(h_tm[:, tb, :], x_t[:, tb, :])
                    h1_fm = transpose_tm_to_fm(h_tm, f"h1fm{l}")

                # ---- V token-major (+bias) -> send buffer ----
                nc.enter_named_scope(f"L{l:02d}_b_v", False)
                bv_bc = ap.tile([P, D], F32, tag="bv_bc", bufs=1, name=f"bvbc{l}")
                nc.gpsimd.partition_broadcast(bv_bc[:], bv_row[:])
                v_send = ap.tile([P, NB_T, D], BF16, tag="vsend", bufs=1, name=f"vsend{l}")
                for tb in range(NB_T):
                    vp = psA.tile([P, D], F32, tag="psA", name=f"vps{l}_{tb}")
                    for n0, n1 in _regions():
                        for kb in range(NB_D):
                            nc.tensor.matmul(
                                vp[:, n0:n1],
                                h1_fm[:, kb, tb * P:(tb + 1) * P],
                                wv[:, kb, n0:n1],
                                start=(kb == 0), stop=(kb == NB_D - 1),
                            )
                    nc.vector.tensor_tensor(v_send[:, tb, :], vp[:], bv_bc[:], OP.add)

                nc.leave_named_scope(f"L{l:02d}_b_v", None, False)
                nc.enter_named_scope(f"L{l:02d}_c_ag", False)
                vsend_a = dp.tile([2, P, D], BF16, tag="vsend_a", name=f"vsa{l}")
                vsend_b = dp.tile([2, P, D], BF16, tag="vsend_b", name=f"vsb{l}")
                for tb in range(2):
                    nc.sync.dma_start(vsend_a[tb], v_send[:, tb, :])
                vrecv_a = dp.tile([4, P, D], BF16, tag="vrecv_a", name=f"vra{l}")
                nc.gpsimd.collective_compute(
                    "AllGather", OP.bypass, replica_groups=REPLICA_GROUPS,
                    ins=[vsend_a[:]], outs=[vrecv_a[:]],
                )
                for tb in range(2):
                    nc.sync.dma_start(vsend_b[tb], v_send[:, 2 + tb, :])
                vrecv_b = dp.tile([4, P, D], BF16, tag="vrecv_b", name=f"vrb{l}")
                nc.gpsimd.collective_compute(
                    "AllGather", OP.bypass, replica_groups=REPLICA_GROUPS,
                    ins=[vsend_b[:]], outs=[vrecv_b[:]],
                )
                nc.leave_named_scope(f"L{l:02d}_c_ag", None, False)
                nc.enter_named_scope(f"L{l:02d}_d_q", False)
                # ---- Q (overlaps the AllGather) ----
                for m in range(NB_D):
                    qp = psB.tile([P, T_OWN], F32, tag="psB", name=f"qps{l}_{m}")
                    for kb in range(NB_D):
                        nc.tensor.matmul(
                            qp[:], wq[:, kb, m * P:(m + 1) * P], h1_fm[:, kb, :],
                            start=(kb == 0), stop=(kb == NB_D - 1),
                        )
                    for hh in range(2):
                        r0 = 64 * hh
                        nc.vector.tensor_scalar(
                            out=q_par[hh][r0:r0 + 64, m, :], in0=qp[r0:r0 + 64, :],
                            scalar1=bq_all[r0:r0 + 64, l, m:m + 1], scalar2=None,
                            op0=OP.add,
                        )

                # prefetch Wo while attention runs
                wo = wp.tile([P, NB_D, D], BF16, tag="wbf", bufs=5, name=f"wo{l}")
                nc.sync.dma_start(wo[:], wo_d[l].rearrange("p (k n) -> p k n", n=D))

                nc.leave_named_scope(f"L{l:02d}_d_q", None, False)
                nc.enter_named_scope(f"L{l:02d}_e_vrecv", False)
                # ---- receive V: augmented token-major + feature-major ----
                v_aug = ap.tile([P, NB_K, H * 65], BF16, tag="v_aug", bufs=1, name=f"vaug{l}")
                va4 = v_aug.rearrange("p k (h c) -> p k h c", c=65)
                v_fm = ap.tile([P, NB_D, NB_K * P], BF16, tag="v_fm", bufs=1, name=f"vfm{l}")
                # group a: rows [s, t] -> global kb = 4s + t ; group b: kb = 4s + 2 + t
                for gi, vr in enumerate((vrecv_a, vrecv_b)):
                    for s in range(2):
                        for t in range(2):
                            kb = 4 * s + 2 * gi + t
                            nc.vector.memset(va4[:, kb, :, 64:65], 1.0)
                            nc.sync.dma_start(
                                va4[:, kb, :, 0:64],
                                vr[2 * s + t].rearrange("p (h c) -> p h c", c=64),
                            )
                    vr_flat = vr.rearrange("k p n -> (k p) n")
                    for db in range(NB_D):
                        for s in range(2):
                            kb0 = 4 * s + 2 * gi
                            nc.sync.dma_start_transpose(
                                v_fm[:, db, kb0 * P:(kb0 + 2) * P],
                                vr_flat[256 * s:256 * (s + 1), db * P:(db + 1) * P],
                            )
                nc.leave_named_scope(f"L{l:02d}_e_vrecv", None, False)
                nc.enter_named_scope(f"L{l:02d}_f_attn", False)
                # ---- attention (head pairs packed into disjoint PE row groups) ----
                ctx_n = ap.tile([P, NB_D, T_OWN], BF16, tag="ctx_n", bufs=1, name=f"ctxn{l}")
                for db in range(NB_D):
                    hpair = (2 * db, 2 * db + 1)
                    ctxp = [
                        psB.tile([65, T_OWN], F32, tag="psB", name=f"ctxp{l}_{h}")
                        for h in hpair
                    ]
                    prev_ex = None
                    prev_pairs = None
                    KB_PAIRS = [(0, 1), (4, 5), (2, 3), (6, 7)]
                    for j in range(NB_K // 2):
                        spv = [
                            psA.tile([P, 1024], F32, tag="psA", name=f"sc{l}_{db}_{j}_{hh}")
                            for hh in range(2)
                        ]
                        for o in range(2):
                            kb = KB_PAIRS[j][o]
                            for hh in range(2):
                                nc.tensor.matmul(
                                    spv[hh][:, o * 512:(o + 1) * 512],
                                    v_fm[:, db, kb * P:(kb + 1) * P],
                                    q_par[hh][:, db, :],
                                    start=True, stop=True,
                                )
                        if prev_ex is not None:
                            jm = j - 1
                            for hh in range(2):
                                for o in range(2):
                                    kb = prev_pairs[o]
                                    nc.tensor.matmul(
                                        ctxp[hh][:],
                                        v_aug[:, kb, 65 * hpair[hh]:65 * hpair[hh] + 65],
                                        prev_ex[hh][:, o, :],
                                        start=(jm == 0 and o == 0), stop=False,
                                    )
                        ex = [
                            ap.tile([P, 2, 512], BF16, tag="expT", bufs=5,
                                    name=f"ex{l}_{db}_{j}_{hh}")
                            for hh in range(2)
                        ]
                        for hh in range(2):
                            nc.scalar.activation(
                                ex[hh].rearrange("p a b -> p (a b)"), spv[hh][:],
                                AF.Exp, scale=SCALE,
                            )
                        prev_ex = ex
                        prev_pairs = KB_PAIRS[j]
                    jm = NB_K // 2 - 1
                    for hh in range(2):
                        for o in range(2):
                            kb = prev_pairs[o]
                            nc.tensor.matmul(
                                ctxp[hh][:],
                                v_aug[:, kb, 65 * hpair[hh]:65 * hpair[hh] + 65],
                                prev_ex[hh][:, o, :],
                                start=False, stop=(o == 1),
                            )
                    for hh in range(2):
                        h = hpair[hh]
                        r0 = 64 * hh
                        zinv = ap.tile([1, T_OWN], F32, tag="zinv", bufs=2, name=f"zi{l}_{h}")
                        nc.vector.reciprocal(zinv[:], ctxp[hh][64:65, :])
                        zbc = ap.tile([P, T_OWN], F32, tag="zbc", bufs=2, name=f"zb{l}_{h}")
                        nc.gpsimd.partition_broadcast(zbc[:], zinv[:])
                        nc.vector.tensor_tensor(
                            ctx_n[r0:r0 + 64, db, :], ctxp[hh][0:64, :], zbc[0:64, :],
                            OP.mult,
                        )
                nc.leave_named_scope(f"L{l:02d}_f_attn", None, False)
                nc.enter_named_scope(f"L{l:02d}_g_wo", False)
                # ---- Wo + residual (+ LN2 per block as it completes) ----
                skip = sp.tile([P, NB_T, D], F32R, tag="stream", name=f"skip{l}")
                h_tm2 = ap.tile([P, NB_T, D], F32R, tag="h_tm", bufs=1, name=f"h2tm{l}")
                for lb in range(NB_T):
                    wps = psA.tile([P, D], F32, tag="psA", name=f"wops{l}_{lb}")
                    for n0, n1 in _regions():
                        for kb in range(NB_D):
                            nc.tensor.matmul(
                                wps[:, n0:n1],
                                ctx_n[:, kb, lb * P:(lb + 1) * P],
                                wo[:, kb, n0:n1],
                                start=(kb == 0), stop=False,
                            )
                        nc.tensor.matmul(
                            wps[:, n0:n1], ones1[:], bo_row[:, n0:n1],
                            start=False, stop=True,
                        )
                    nc.vector.tensor_tensor(skip[:, lb, :], x_t[:, lb, :], wps[:], OP.add)
                    ln(h_tm2[:, lb, :], skip[:, lb, :])
                nc.leave_named_scope(f"L{l:02d}_g_wo", None, False)
                nc.enter_named_scope(f"L{l:02d}_h_ln2", False)
                # ---- LN2 transpose ----
                h2_fm = transpose_tm_to_fm(h_tm2, f"h2fm{l}")
                nc.leave_named_scope(f"L{l:02d}_h_ln2", None, False)
                nc.enter_named_scope(f"L{l:02d}_i_ff", False)
                # ---- FFN: FF1 (all 24 hidden blocks) then FF2 accumulated in PSUM ----
                g_all = ap.tile([P, NB_FF, T_OWN], BF16, tag="g", bufs=1, name=f"g{l}")
                w2cs = []
                for ck in range(4):
                    w1c = wp.tile([P, NB_D, D], BF16, tag="wbf", bufs=5, name=f"w1c{l}_{ck}")
                    nc.sync.dma_start(w1c[:], w1_d[l, ck].rearrange("p (k n) -> p k n", n=D))
                    w2c = wp.tile([P, NB_D, D], BF16, tag="wbf", bufs=5, name=f"w2c{l}_{ck}")
                    nc.sync.dma_start(w2c[:], w2_d[l, ck].rearrange("p (k n) -> p k n", n=D))
                    w2cs.append(w2c)
                    for mm in range(NB_D):
                        fp = psB.tile([P, T_OWN], F32, tag="psB", name=f"f1ps{l}_{ck}_{mm}")
                        for kb in range(NB_D):
                            nc.tensor.matmul(
                                fp[:], w1c[:, kb, mm * P:(mm + 1) * P], h2_fm[:, kb, :],
                                start=(kb == 0), stop=(kb == NB_D - 1),
                            )
                        nc.scalar.activation(
                            g_all[:, 6 * ck + mm, :], fp[:], AF.Gelu,
                            bias=b1_all[:, l, 6 * ck + mm:6 * ck + mm + 1], scale=1.0,
                        )
                for half in range(2):
                    f2s = []
                    for lb in (2 * half, 2 * half + 1):
                        f2 = psA.tile([P, D], F32, tag="psA", name=f"f2ps{l}_{lb}")
                        f2s.append(f2)
                        for n0, n1 in _regions():
                            for ck in range(4):
                                for mm in range(NB_D):
                                    nc.tensor.matmul(
                                        f2[:, n0:n1],
                                        g_all[:, 6 * ck + mm, lb * P:(lb + 1) * P],
                                        w2cs[ck][:, mm, n0:n1],
                                        start=(ck == 0 and mm == 0), stop=False,
                                    )
                            nc.tensor.matmul(
                                f2[:, n0:n1], ones1[:], b2_row[:, n0:n1],
                                start=False, stop=True,
                            )
                    for i, lb in enumerate((2 * half, 2 * half + 1)):
                        nc.vector.tensor_tensor(
                            skip[:, lb, :], skip[:, lb, :], f2s[i][:], OP.add,
                        )
                nc.leave_named_scope(f"L{l:02d}_i_ff", None, False)
                x_t = skip

            nc.sync.dma_start(out_d[:], x_t[:])
    nc.compile()
    return nc


def _preprocess(inputs, n_layers):
    """Fold LN affine into projections; lay out weights for tile DMA."""
    f32 = np.float32
    L = n_layers
    Wq = np.asarray(inputs["Wq"], f32)[:L]
    Wv = np.asarray(inputs["Wv"], f32)[:L]
    Wo = np.asarray(inputs["Wo"], f32)[:L]
    W1 = np.asarray(inputs["W1"], f32)[:L]
    W2 = np.asarray(inputs["W2"], f32)[:L]
    g1 = np.asarray(inputs["ln1_g"], f32)[:L]
    b1ln = np.asarray(inputs["ln1_b"], f32)[:L]
    g2 = np.asarray(inputs["ln2_g"], f32)[:L]
    b2ln = np.asarray(inputs["ln2_b"], f32)[:L]
    bq = np.asarray(inputs["bq"], f32)[:L]
    bv = np.asarray(inputs["bv"], f32)[:L]
    bo = np.asarray(inputs["bo"], f32)[:L]
    b1 = np.asarray(inputs["b1"], f32)[:L]
    b2 = np.asarray(inputs["b2"], f32)[:L]

    Wq_eff = g1[:, :, None] * Wq
    bq_eff = bq + np.einsum("ld,ldo->lo", b1ln, Wq)
    Wv_eff = g1[:, :, None] * Wv
    bv_eff = bv + np.einsum("ld,ldo->lo", b1ln, Wv)
    W1_eff = g2[:, :, None] * W1
    b1_eff = b1 + np.einsum("ld,ldo->lo", b2ln, W1)

    def fm_weight(W):  # [L, D, D] -> [L, 128, 6*768] with [p, k, n]
        return np.ascontiguousarray(
            W.reshape(L, NB_D, P, D).transpose(0, 2, 1, 3).reshape(L, P, NB_D * D)
        )

    bf = ml_dtypes.bfloat16
    wq_h = fm_weight(Wq_eff).astype(bf)
    wv_h = fm_weight(Wv_eff).astype(bf)
    wo_h = fm_weight(Wo).astype(bf)
    w1_h = np.ascontiguousarray(
        W1_eff.reshape(L, NB_D, P, 4, D).transpose(0, 3, 2, 1, 4).reshape(L, 4, P, NB_D * D)
    ).astype(bf)
    w2_h = np.ascontiguousarray(
        W2.reshape(L, 4, NB_D, P, D).transpose(0, 1, 3, 2, 4).reshape(L, 4, P, NB_D * D)
    ).astype(ml_dtypes.bfloat16)
    bq_h = np.ascontiguousarray(bq_eff.reshape(L, NB_D, P).transpose(2, 0, 1))
    b1_h = np.ascontiguousarray(b1_eff.reshape(L, NB_FF, P).transpose(2, 0, 1))

    return {
        "wq": wq_h, "wv": wv_h, "wo": wo_h, "w1": w1_h, "w2": w2_h,
        "bq": bq_h, "b1": b1_h,
        "bv_row": np.ascontiguousarray(bv_eff[None]),
        "bo_row": np.ascontiguousarray(bo[None]).astype(bf),
        "b2_row": np.ascontiguousarray(b2[None]).astype(bf),
        "ident32": np.eye(P, dtype=f32),
        "identbf": np.eye(P).astype(ml_dtypes.bfloat16),
        "ones1": np.ones((1, P)).astype(bf),
    }


def kernel(**inputs) -> np.ndarray:
    n_layers = N_LAYERS
    key = ("nc", n_layers)
    if key not in _cached:
        _cached[key] = build(n_layers)
    nc = _cached[key]

    shared = _preprocess(inputs, n_layers)
    x = np.asarray(inputs["x"], np.float32)  # [4, 1024, 768]
    B, T, _ = x.shape

    in_maps = []
    for c in range(8):
        b, half = c // 2, c % 2
        x_own = x[b, half * T_OWN:(half + 1) * T_OWN]          # [512, 768]
        x_tile = np.ascontiguousarray(
            x_own.reshape(NB_T, P, D).transpose(1, 0, 2)        # [128, 4, 768]
        )
        in_maps.append({**shared, "x": x_tile})

    trace = bool(int(os.environ.get("KERNEL_TRACE", "0")))
    if trace:
        _register_ntff_hook()
    res = run_bass_kernel_spmd(nc, in_maps, core_ids=list(range(8)), trace=trace)
    global _last_results
    _last_results = res

    out = np.empty((B, T, D), dtype=np.float32)
    for c in range(8):
        b, half = c // 2, c % 2
        o = res.results[c]["out"]                               # [128, 4, 768]
        out[b, half * T_OWN:(half + 1) * T_OWN] = (
            o.transpose(1, 0, 2).reshape(T_OWN, D)
        )
    return out


# revision 17
# speedup vs baseline: 1.0531x; 1.0531x over previous
"""Trainium2 Bass kernel for nn_Encoder_506806141403.

12-layer transformer encoder (D=768, H=12, FF=3072) with the quirk that
attention scores use Q vs V (no K projection) and scale by D**-0.5.

Sharding: 8 cores = 4 batch elements x 2 sequence halves. Each core owns
512 query rows of one batch element. Per layer, each core computes its half's
V projection (token-major, bf16) and the pair exchanges halves via a 2-rank
AllGather, so every core holds all 1024 keys/values of its batch element.
Everything else (LN, Q, scores, softmax, ctx, Wo, FFN) is computed per-core
on its own 512 rows. Matmuls run in float32r (full-rate fp32 on the PE) for
the residual/FFN path and bf16 for the attention-probability path.

Layout notes:
- Residual stream is token-major f32r SBUF tiles [128, 4, 768].
- Matmul inputs are feature-major (contraction dim on partitions); activations
  are transposed via PE-transpose where needed.
- Softmax runs on key-major scoresT [k, q] psum; Z is obtained by augmenting
  the token-major V with a ones column (M=65 matmuls); normalization uses a
  reciprocal + gpsimd partition_broadcast + one DVE multiply per head.
- LN gains/biases are folded into the following projection weights on the
  host; remaining per-channel biases use per-partition ACT bias (feature-
  major outputs) or K=1 outer-product matmuls (token-major outputs).
"""
import os
import sys

sys.path.insert(0, "/opt/trn_rl_repo")

import numpy as np
import ml_dtypes

import concourse.bass as bass
from concourse import bacc
import concourse.tile as tile
from concourse import mybir
from concourse.bass_utils import run_bass_kernel_spmd
from concourse import bass_utils as _bu

if int(os.environ.get("KERNEL_LDW_OPT", "0")):
    _orig_run_command = _bu.run_command

    def _patched_run_command(argv, **kwargs):
        argv = [a.replace("--enable-ldw-opt=false", "--enable-ldw-opt=true")
                if isinstance(a, str) else a for a in argv]
        return _orig_run_command(argv, **kwargs)

    _bu.run_command = _patched_run_command

P = 128
D = 768
H = 12
DH = 64
FF = 3072
NB_D = 6          # D / P
NB_T = 4          # own tokens 512 / P
NB_K = 8          # full tokens 1024 / P
NB_FF = 24        # FF / P
T_OWN = 512
SCALE = float(D) ** -0.5
LN_EPS = 1e-5
N_LAYERS = int(os.environ.get("KERNEL_N_LAYERS", "12"))

F32 = mybir.dt.float32
F32R = mybir.dt.float32r
BF16 = mybir.dt.bfloat16
AF = mybir.ActivationFunctionType
OP = mybir.AluOpType

REPLICA_GROUPS = [[0, 1], [2, 3], [4, 5], [6, 7]]

_cached = {}
_last_results = None


def _register_ntff_hook():
    """Register the axon NTFF profile hook (for trace=True exec timing)."""
    import types
    try:
        import antenv.axon_hooks  # noqa: F401
        return
    except ImportError:
        pass
    try:
        from trn_agent_boot.trn_boot import _ntff_profile_via_ctypes
        import antenv
        hook = _ntff_profile_via_ctypes("/opt/axon/libaxon_pjrt.so")
        mod = types.ModuleType("antenv.axon_hooks")
        mod.get_axon_ntff_profile_hook = lambda: hook
        mod.set_axon_ntff_profile_hook = lambda h: None
        sys.modules["antenv.axon_hooks"] = mod
        antenv.axon_hooks = mod
    except Exception:
        pass


def _regions():
    return ((0, 512), (512, 768))


def build(n_layers=N_LAYERS):
    nc = bacc.Bacc(None, target_bir_lowering=False, num_devices=8)
    L = n_layers

    x_d = nc.dram_tensor("x", [P, NB_T, D], F32R, kind="ExternalInput")
    wq_d = nc.dram_tensor("wq", [L, P, NB_D * D], BF16, kind="ExternalInput")
    wv_d = nc.dram_tensor("wv", [L, P, NB_D * D], BF16, kind="ExternalInput")
    wo_d = nc.dram_tensor("wo", [L, P, NB_D * D], BF16, kind="ExternalInput")
    w1_d = nc.dram_tensor("w1", [L, 4, P, NB_D * D], BF16, kind="ExternalInput")
    w2_d = nc.dram_tensor("w2", [L, 4, P, NB_D * D], BF16, kind="ExternalInput")
    bq_d = nc.dram_tensor("bq", [P, L, NB_D], F32, kind="ExternalInput")
    b1_d = nc.dram_tensor("b1", [P, L, NB_FF], F32, kind="ExternalInput")
    bvr_d = nc.dram_tensor("bv_row", [1, L, D], F32, kind="ExternalInput")
    bor_d = nc.dram_tensor("bo_row", [1, L, D], BF16, kind="ExternalInput")
    b2r_d = nc.dram_tensor("b2_row", [1, L, D], BF16, kind="ExternalInput")
    id32_d = nc.dram_tensor("ident32", [P, P], F32R, kind="ExternalInput")
    idbf_d = nc.dram_tensor("identbf", [P, P], BF16, kind="ExternalInput")
    ones_d = nc.dram_tensor("ones1", [1, P], BF16, kind="ExternalInput")
    out_d = nc.dram_tensor("out", [P, NB_T, D], F32R, kind="ExternalOutput")

    with tile.TileContext(nc) as tc:
        with (
            tc.tile_pool(name="state", bufs=1) as st,
            tc.tile_pool(name="stream", bufs=2) as sp,
            tc.tile_pool(name="acts", bufs=1) as ap,
            tc.tile_pool(name="wpool", bufs=2) as wp,
            tc.tile_pool(name="psA", bufs=2, space="PSUM") as psA,
            tc.tile_pool(name="psB", bufs=4, space="PSUM") as psB,
            tc.tile_pool(name="dram", bufs=2, space="DRAM") as dp,
        ):
            # ---- constants ----
            id32 = st.tile([P, P], F32R)
            idbf = st.tile([P, P], BF16)
            ones1 = st.tile([1, P], BF16)
            bq_all = st.tile([P, L, NB_D], F32)
            b1_all = st.tile([P, L, NB_FF], F32)
            eps_t = st.tile([P, 1], F32)
            q_par = [st.tile([P, NB_D, T_OWN], BF16, name=f"qpar{p}") for p in range(2)]
            nc.vector.memset(q_par[0][:], 0.0)
            nc.vector.memset(q_par[1][:], 0.0)
            nc.sync.dma_start(id32[:], id32_d[:])
            nc.sync.dma_start(idbf[:], idbf_d[:])
            nc.sync.dma_start(ones1[:], ones_d[:])
            nc.sync.dma_start(bq_all[:], bq_d[:])
            nc.sync.dma_start(b1_all[:], b1_d[:])
            nc.vector.memset(eps_t[:], LN_EPS)

            def ln(out_ap, in_ap):
                """LayerNorm (normalize only) along free axis of [128, 768]."""
                t = ap.tile([P, 32], F32, tag="lnscratch", bufs=3, name="lnt")
                stt = t[:, 0:18].rearrange("p (g s) -> p g s", s=6)
                xg = in_ap.rearrange("p (g d) -> p g d", g=3)
                for g in range(3):
                    nc.vector.bn_stats(stt[:, g, :], xg[:, g, :])
                mv = t[:, 18:20]
                nc.vector.bn_aggr(mv[:], stt[:])
                nc.scalar.activation(t[:, 20:21], mv[:, 1:2], AF.Ln, bias=eps_t[:], scale=1.0)
                nc.scalar.activation(t[:, 21:22], t[:, 20:21], AF.Exp, scale=-0.5)
                nc.vector.tensor_scalar(
                    out=out_ap, in0=in_ap, scalar1=mv[:, 0:1], scalar2=t[:, 21:22],
                    op0=OP.subtract, op1=OP.mult,
                )

            def transpose_tm_to_fm(h_tm, name):
                """[128, 4, 768] token-major -> [128, 6, 512] feature-major."""
                h_fm = ap.tile([P, NB_D, T_OWN], BF16, tag="h_fm", bufs=1, name=name)
                for db in range(NB_D):
                    trp = psB.tile([P, T_OWN], F32R, tag="psB", name=f"{name}_tr{db}")
                    for tb in range(NB_T):
                        nc.tensor.transpose(
                            trp[:, tb * P:(tb + 1) * P],
                            h_tm[:, tb, db * P:(db + 1) * P], id32[:],
                        )
                    nc.vector.tensor_copy(h_fm[:, db, :], trp[:])
                return h_fm

            # ---- initial stream ----
            x_t = sp.tile([P, NB_T, D], F32R, tag="stream", name="x0")
            nc.sync.dma_start(x_t[:], x_d[:])

            for l in range(L):
                # ---- weights for this layer ----
                wq = wp.tile([P, NB_D, D], BF16, tag="wbf", bufs=5, name=f"wq{l}")
                nc.sync.dma_start(wq[:], wq_d[l].rearrange("p (k n) -> p k n", n=D))
                wv = wp.tile([P, NB_D, D], BF16, tag="wbf", bufs=5, name=f"wv{l}")
                nc.sync.dma_start(wv[:], wv_d[l].rearrange("p (k n) -> p k n", n=D))
                bv_row = ap.tile([1, D], F32, tag="bvrow", bufs=2, name=f"bvr{l}")
                nc.sync.dma_start(bv_row[:], bvr_d[:, l, :])
                bo_row = ap.tile([1, D], BF16, tag="borow", bufs=2, name=f"bor{l}")
                nc.sync.dma_start(bo_row[:], bor_d[:, l, :])
                b2_row = ap.tile([1, D], BF16, tag="b2row", bufs=2, name=f"b2r{l}")
                nc.sync.dma_start(b2_row[:], b2r_d[:, l, :])

                # ---- LN1 + transpose ----
                with nc.named_scope(f"L{l:02d}_a_ln1"):
                    h_tm = ap.tile([P, NB_T, D], F32R, tag="h_tm", bufs=1, name=f"h1tm{l}")
                    for tb in range(NB_T):
                        ln(h_tm[:, tb, :], x_t[:, tb, :])
                    h1_fm = transpose_tm_to_fm(h_tm, f"h1fm{l}")

                # ---- V token-major (+bias) -> send buffer ----
                nc.enter_named_scope(f"L{l:02d}_b_v", False)
                bv_bc = ap.tile([P, D], F32, tag="bv_bc", bufs=1, name=f"bvbc{l}")
                nc.gpsimd.partition_broadcast(bv_bc[:], bv_row[:])
                v_send = ap.tile([P, NB_T, D], BF16, tag="vsend", bufs=1, name=f"vsend{l}")
                for tb in range(NB_T):
                    vp = psA.tile([P, D], F32, tag="psA", name=f"vps{l}_{tb}")
                    for n0, n1 in _regions():
                        for kb in range(NB_D):
                            nc.tensor.matmul(
                                vp[:, n0:n1],
                                h1_fm[:, kb, tb * P:(tb + 1) * P],
                                wv[:, kb, n0:n1],
                                start=(kb == 0), stop=(kb == NB_D - 1),
                            )
                    nc.vector.tensor_tensor(v_send[:, tb, :], vp[:], bv_bc[:], OP.add)

                nc.leave_named_scope(f"L{l:02d}_b_v", None, False)
                nc.enter_named_scope(f"L{l:02d}_c_ag", False)
                vsend_dr = dp.tile([NB_T, P, D], BF16, tag="vsend_d", name=f"vsdr{l}")
                for tb in range(NB_T):
                    nc.sync.dma_start(vsend_dr[tb], v_send[:, tb, :])
                vrecv_dr = dp.tile([NB_K, P, D], BF16, tag="vrecv_d", name=f"vrdr{l}")
                nc.gpsimd.collective_compute(
                    "AllGather", OP.bypass, replica_groups=REPLICA_GROUPS,
                    ins=[vsend_dr[:]], outs=[vrecv_dr[:]],
                )
                nc.leave_named_scope(f"L{l:02d}_c_ag", None, False)
                nc.enter_named_scope(f"L{l:02d}_d_q", False)
                # ---- Q (overlaps the AllGather) ----
                for m in range(NB_D):
                    qp = psB.tile([P, T_OWN], F32, tag="psB", name=f"qps{l}_{m}")
                    for kb in range(NB_D):
                        nc.tensor.matmul(
                            qp[:], wq[:, kb, m * P:(m + 1) * P], h1_fm[:, kb, :],
                            start=(kb == 0), stop=(kb == NB_D - 1),
                        )
                    for hh in range(2):
                        r0 = 64 * hh
                        nc.vector.tensor_scalar(
                            out=q_par[hh][r0:r0 + 64, m, :], in0=qp[r0:r0 + 64, :],
                            scalar1=bq_all[r0:r0 + 64, l, m:m + 1], scalar2=None,
                            op0=OP.add,
                        )

                # prefetch Wo while attention runs
                wo = wp.tile([P, NB_D, D], BF16, tag="wbf", bufs=5, name=f"wo{l}")
                nc.sync.dma_start(wo[:], wo_d[l].rearrange("p (k n) -> p k n", n=D))

                nc.leave_named_scope(f"L{l:02d}_d_q", None, False)
                nc.enter_named_scope(f"L{l:02d}_e_vrecv", False)
                # ---- receive V: augmented token-major + feature-major ----
                v_aug = ap.tile([P, NB_K, H * 65], BF16, tag="v_aug", bufs=1, name=f"vaug{l}")
                va4 = v_aug.rearrange("p k (h c) -> p k h c", c=65)
                for kb in range(NB_K):
                    nc.vector.memset(va4[:, kb, :, 64:65], 1.0)
                    nc.sync.dma_start(
                        va4[:, kb, :, 0:64],
                        vrecv_dr[kb].rearrange("p (h c) -> p h c", c=64),
                    )
                v_fm = ap.tile([P, NB_D, NB_K * P], BF16, tag="v_fm", bufs=1, name=f"vfm{l}")
                vr_flat = vrecv_dr.rearrange("k p n -> (k p) n")
                for db in range(NB_D):
                    nc.sync.dma_start_transpose(
                        v_fm[:, db, :], vr_flat[:, db * P:(db + 1) * P]
                    )
                nc.leave_named_scope(f"L{l:02d}_e_vrecv", None, False)
                nc.enter_named_scope(f"L{l:02d}_f_attn", False)
                # ---- attention (head pairs packed into disjoint PE row groups) ----
                ctx_n = ap.tile([P, NB_D, T_OWN], BF16, tag="ctx_n", bufs=1, name=f"ctxn{l}")
                for db in range(NB_D):
                    hpair = (2 * db, 2 * db + 1)
                    ctxp = [
                        psB.tile([65, T_OWN], F32, tag="psB", name=f"ctxp{l}_{h}")
                        for h in hpair
                    ]
                    prev_ex = None
                    prev_pairs = None
                    KB_PAIRS = [(0, 1), (2, 3), (4, 5), (6, 7)]
                    for j in range(NB_K // 2):
                        spv = [
                            psA.tile([P, 1024], F32, tag="psA", name=f"sc{l}_{db}_{j}_{hh}")
                            for hh in range(2)
                        ]
                        for o in range(2):
                            kb = KB_PAIRS[j][o]
                            for hh in range(2):
                                nc.tensor.matmul(
                                    spv[hh][:, o * 512:(o + 1) * 512],
                                    v_fm[:, db, kb * P:(kb + 1) * P],
                                    q_par[hh][:, db, :],
                                    start=True, stop=True,
                                )
                        if prev_ex is not None:
                            jm = j - 1
                            for hh in range(2):
                                for o in range(2):
                                    kb = prev_pairs[o]
                                    nc.tensor.matmul(
                                        ctxp[hh][:],
                                        v_aug[:, kb, 65 * hpair[hh]:65 * hpair[hh] + 65],
                                        prev_ex[hh][:, o, :],
                                        start=(jm == 0 and o == 0), stop=False,
                                    )
                        ex = [
                            ap.tile([P, 2, 512], BF16, tag="expT", bufs=5,
                                    name=f"ex{l}_{db}_{j}_{hh}")
                            for hh in range(2)
                        ]
                        for hh in range(2):
                            nc.scalar.activation(
                                ex[hh].rearrange("p a b -> p (a b)"), spv[hh][:],
                                AF.Exp, scale=SCALE,
                            )
                        prev_ex = ex
                        prev_pairs = KB_PAIRS[j]
                    jm = NB_K // 2 - 1
                    for hh in range(2):
                        for o in range(2):
                            kb = prev_pairs[o]
                            nc.tensor.matmul(
                                ctxp[hh][:],
                                v_aug[:, kb, 65 * hpair[hh]:65 * hpair[hh] + 65],
                                prev_ex[hh][:, o, :],
                                start=False, stop=(o == 1),
                            )
                    for hh in range(2):
                        h = hpair[hh]
                        r0 = 64 * hh
                        zinv = ap.tile([1, T_OWN], F32, tag="zinv", bufs=2, name=f"zi{l}_{h}")
                        nc.vector.reciprocal(zinv[:], ctxp[hh][64:65, :])
                        zbc = ap.tile([P, T_OWN], F32, tag="zbc", bufs=2, name=f"zb{l}_{h}")
                        nc.gpsimd.partition_broadcast(zbc[:], zinv[:])
                        nc.vector.tensor_tensor(
                            ctx_n[r0:r0 + 64, db, :], ctxp[hh][0:64, :], zbc[0:64, :],
                            OP.mult,
                        )
                nc.leave_named_scope(f"L{l:02d}_f_attn", None, False)
                nc.enter_named_scope(f"L{l:02d}_g_wo", False)
                # ---- Wo + residual (+ LN2 per block as it completes) ----
                skip = sp.tile([P, NB_T, D], F32R, tag="stream", name=f"skip{l}")
                h_tm2 = ap.tile([P, NB_T, D], F32R, tag="h_tm", bufs=1, name=f"h2tm{l}")
                for lb in range(NB_T):
                    wps = psA.tile([P, D], F32, tag="psA", name=f"wops{l}_{lb}")
                    for n0, n1 in _regions():
                        for kb in range(NB_D):
                            nc.tensor.matmul(
                                wps[:, n0:n1],
                                ctx_n[:, kb, lb * P:(lb + 1) * P],
                                wo[:, kb, n0:n1],
                                start=(kb == 0), stop=False,
                            )
                        nc.tensor.matmul(
                            wps[:, n0:n1], ones1[:], bo_row[:, n0:n1],
                            start=False, stop=True,
                        )
                    nc.vector.tensor_tensor(skip[:, lb, :], x_t[:, lb, :], wps[:], OP.add)
                    ln(h_tm2[:, lb, :], skip[:, lb, :])
                nc.leave_named_scope(f"L{l:02d}_g_wo", None, False)
                nc.enter_named_scope(f"L{l:02d}_h_ln2", False)
                # ---- LN2 transpose ----
                h2_fm = transpose_tm_to_fm(h_tm2, f"h2fm{l}")
                nc.leave_named_scope(f"L{l:02d}_h_ln2", None, False)
                nc.enter_named_scope(f"L{l:02d}_i_ff", False)
                # ---- FFN: FF1 (all 24 hidden blocks) then FF2 accumulated in PSUM ----
                g_all = ap.tile([P, NB_FF, T_OWN], BF16, tag="g", bufs=1, name=f"g{l}")
                w2cs = []
                for ck in range(4):
                    w1c = wp.tile([P, NB_D, D], BF16, tag="wbf", bufs=5, name=f"w1c{l}_{ck}")
                    nc.sync.dma_start(w1c[:], w1_d[l, ck].rearrange("p (k n) -> p k n", n=D))
                    w2c = wp.tile([P, NB_D, D], BF16, tag="wbf", bufs=5, name=f"w2c{l}_{ck}")
                    nc.sync.dma_start(w2c[:], w2_d[l, ck].rearrange("p (k n) -> p k n", n=D))
                    w2cs.append(w2c)
                    for mm in range(NB_D):
                        fp = psB.tile([P, T_OWN], F32, tag="psB", name=f"f1ps{l}_{ck}_{mm}")
                        for kb in range(NB_D):
                            nc.tensor.matmul(
                                fp[:], w1c[:, kb, mm * P:(mm + 1) * P], h2_fm[:, kb, :],
                                start=(kb == 0), stop=(kb == NB_D - 1),
                            )
                        nc.scalar.activation(
                            g_all[:, 6 * ck + mm, :], fp[:], AF.Gelu,
                            bias=b1_all[:, l, 6 * ck + mm:6 * ck + mm + 1], scale=1.0,
                        )
                for half in range(2):
                    f2s = []
                    for lb in (2 * half, 2 * half + 1):
                        f2 = psA.tile([P, D], F32, tag="psA", name=f"f2ps{l}_{lb}")
                        f2s.append(f2)
                        for n0, n1 in _regions():
                            for ck in range(4):
                                for mm in range(NB_D):
                                    nc.tensor.matmul(
                                        f2[:, n0:n1],
                                        g_all[:, 6 * ck + mm, lb * P:(lb + 1) * P],
                                        w2cs[ck][:, mm, n0:n1],
                                        start=(ck == 0 and mm == 0), stop=False,
                                    )
                            nc.tensor.matmul(
                                f2[:, n0:n1], ones1[:], b2_row[:, n0:n1],
                                start=False, stop=True,
                            )
                    for i, lb in enumerate((2 * half, 2 * half + 1)):
                        nc.vector.tensor_tensor(
                            skip[:, lb, :], skip[:, lb, :], f2s[i][:], OP.add,
                        )
                nc.leave_named_scope(f"L{l:02d}_i_ff", None, False)
                x_t = skip

            nc.sync.dma_start(out_d[:], x_t[:])
    nc.compile()
    return nc


def _preprocess(inputs, n_layers):
    """Fold LN affine into projections; lay out weights for tile DMA."""
    f32 = np.float32
    L = n_layers
    Wq = np.asarray(inputs["Wq"], f32)[:L]
    Wv = np.asarray(inputs["Wv"], f32)[:L]
    Wo = np.asarray(inputs["Wo"], f32)[:L]
    W1 = np.asarray(inputs["W1"], f32)[:L]
    W2 = np.asarray(inputs["W2"], f32)[:L]
    g1 = np.asarray(inputs["ln1_g"], f32)[:L]
    b1ln = np.asarray(inputs["ln1_b"], f32)[:L]
    g2 = np.asarray(inputs["ln2_g"], f32)[:L]
    b2ln = np.asarray(inputs["ln2_b"], f32)[:L]
    bq = np.asarray(inputs["bq"], f32)[:L]
    bv = np.asarray(inputs["bv"], f32)[:L]
    bo = np.asarray(inputs["bo"], f32)[:L]
    b1 = np.asarray(inputs["b1"], f32)[:L]
    b2 = np.asarray(inputs["b2"], f32)[:L]

    Wq_eff = g1[:, :, None] * Wq
    bq_eff = bq + np.einsum("ld,ldo->lo", b1ln, Wq)
    Wv_eff = g1[:, :, None] * Wv
    bv_eff = bv + np.einsum("ld,ldo->lo", b1ln, Wv)
    W1_eff = g2[:, :, None] * W1
    b1_eff = b1 + np.einsum("ld,ldo->lo", b2ln, W1)

    def fm_weight(W):  # [L, D, D] -> [L, 128, 6*768] with [p, k, n]
        return np.ascontiguousarray(
            W.reshape(L, NB_D, P, D).transpose(0, 2, 1, 3).reshape(L, P, NB_D * D)
        )

    bf = ml_dtypes.bfloat16
    wq_h = fm_weight(Wq_eff).astype(bf)
    wv_h = fm_weight(Wv_eff).astype(bf)
    wo_h = fm_weight(Wo).astype(bf)
    w1_h = np.ascontiguousarray(
        W1_eff.reshape(L, NB_D, P, 4, D).transpose(0, 3, 2, 1, 4).reshape(L, 4, P, NB_D * D)
    ).astype(bf)
    w2_h = np.ascontiguousarray(
        W2.reshape(L, 4, NB_D, P, D).transpose(0, 1, 3, 2, 4).reshape(L, 4, P, NB_D * D)
    ).astype(ml_dtypes.bfloat16)
    bq_h = np.ascontiguousarray(bq_eff.reshape(L, NB_D, P).transpose(2, 0, 1))
    b1_h = np.ascontiguousarray(b1_eff.reshape(L, NB_FF, P).transpose(2, 0, 1))

    return {
        "wq": wq_h, "wv": wv_h, "wo": wo_h, "w1": w1_h, "w2": w2_h,
        "bq": bq_h, "b1": b1_h,
        "bv_row": np.ascontiguousarray(bv_eff[None]),
        "bo_row": np.ascontiguousarray(bo[None]).astype(bf),
        "b2_row": np.ascontiguousarray(b2[None]).astype(bf),
        "ident32": np.eye(P, dtype=f32),
        "identbf": np.eye(P).astype(ml_dtypes.bfloat16),
        "ones1": np.ones((1, P)).astype(bf),
    }


def kernel(**inputs) -> np.ndarray:
    n_layers = N_LAYERS
    key = ("nc", n_layers)
    if key not in _cached:
        _cached[key] = build(n_layers)
    nc = _cached[key]

    shared = _preprocess(inputs, n_layers)
    x = np.asarray(inputs["x"], np.float32)  # [4, 1024, 768]
    B, T, _ = x.shape

    in_maps = []
    for c in range(8):
        b, half = c // 2, c % 2
        x_own = x[b, half * T_OWN:(half + 1) * T_OWN]          # [512, 768]
        x_tile = np.ascontiguousarray(
            x_own.reshape(NB_T, P, D).transpose(1, 0, 2)        # [128, 4, 768]
        )
        in_maps.append({**shared, "x": x_tile})

    trace = bool(int(os.environ.get("KERNEL_TRACE", "0")))
    if trace:
        _register_ntff_hook()
    res = run_bass_kernel_spmd(nc, in_maps, core_ids=list(range(8)), trace=trace)
    global _last_results
    _last_results = res

    out = np.empty((B, T, D), dtype=np.float32)
    for c in range(8):
        b, half = c // 2, c % 2
        o = res.results[c]["out"]                               # [128, 4, 768]
        out[b, half * T_OWN:(half + 1) * T_OWN] = (
            o.transpose(1, 0, 2).reshape(T_OWN, D)
        )
    return out
